# revision 1
# baseline (speedup 1.0000x reference)
"""AdmittanceGNN (3-layer edge-attention GNN) on 8 Trainium2 NeuronCores.

Strategy (dst-sharded):
  - Nodes are sharded into 8 contiguous ranges (6272/core, padded to 50176).
  - Each core owns all edges whose dst falls in its range -> segment sums are
    core-local (no big cross-core reduction).
  - Per layer, each core computes feature tables for ITS nodes:
      u = (x@Wn)@W1a   (dst-side attention term, stays local)
      v = (x@Wn)@W1b   (src-side attention term)
      w = (x@Wn)@We1   (src-side message term)
    [v|w] rows are AllGathered into a full table; per-edge v/w are fetched with
    batched dma_gather (int16 indices, windowed by table halves).
  - Attention: a1 = relu(u[dst]+v[src]); att = sigmoid(sum(a1*w2)).
  - Messages never materialize att*(...) per edge: att is folded into a
    one-hot scatter matrix S_att[e, n] = (n == dst_off(e)) * att(e); segment
    sum = S_att^T @ w_gathered via PE matmuls accumulating in PSUM.
    The edge_attr term factors through k=2: P2 = S_att^T @ ea (per block),
    then block += P2 @ We2.
  - LayerNorm + relu + residual per 128-node block, fp32 residual stream.
"""
import math

import numpy as np
import ml_dtypes

import concourse.bass as bass
import concourse.bacc as bacc
import concourse.tile as tile
import concourse.mybir as mybir
from concourse import bass_utils

P = 128
D = 128
H = 64
LN_EPS = 1e-5

f32 = mybir.dt.float32
bf16 = mybir.dt.bfloat16
i16 = mybir.dt.int16
BF = ml_dtypes.bfloat16

DEBUG_TAPS = False
import os
VARIANT = os.environ.get("KVARIANT", "full")  # full | gonly | nogather | notables

AL = mybir.AluOpType
AF = mybir.ActivationFunctionType


class Cfg:
    def __init__(self, N, E, L, ncores, bpc, window_b=2, half=32768):
        self.N, self.E, self.L, self.ncores = N, E, L, ncores
        self.bpc = bpc                      # blocks of 128 nodes per core
        self.nodes_pc = bpc * P
        self.npad = ncores * self.nodes_pc
        self.window_b = window_b
        self.half = half                    # src-index window split
        assert self.npad >= N
        # windows: list of lists of block indices
        self.windows = [list(range(i, min(i + window_b, bpc)))
                        for i in range(0, bpc, window_b)]


REAL = Cfg(N=50000, E=640000, L=3, ncores=8, bpc=49, window_b=2, half=32768)


# ---------------------------------------------------------------- host prep
def prep_edges(cfg, edge_index):
    """Bucket/sort/pad edges; build per-core slot arrays + shared layout.

    Slot order (identical across cores): for each window w (window_b blocks):
      [lo-seg(b0) | lo-seg(b1) | ... | hi-seg(b0) | hi-seg(b1) | ...]
    each segment padded to a multiple of 128. Slot s -> tile t=s//128,
    partition p=s%128.
    """
    src = np.asarray(edge_index[0], dtype=np.int64)
    dst = np.asarray(edge_index[1], dtype=np.int64)
    E = len(src)
    nc_, bpc, npc = cfg.ncores, cfg.bpc, cfg.nodes_pc

    core = dst // npc
    loc = dst - core * npc
    blk = loc // P
    off = loc % P
    is_lo = src < cfg.half

    # counts per (core, blk, half)
    cnt = np.zeros((nc_, bpc, 2), dtype=np.int64)
    np.add.at(cnt, (core, blk, 1 - is_lo.astype(np.int64)), 1)
    tiles = np.maximum(0, -(-cnt.max(axis=0) // P))        # [bpc, 2] shared
    seg_slots = tiles * P

    # global layout
    seg_start = np.zeros((bpc, 2), dtype=np.int64)
    tile_block = []     # per global tile: block index
    tile_first = []     # is first tile of its block (PSUM start)
    tile_last = []      # is last tile of its block (PSUM stop)
    win_meta = []       # per window: dict
    pos = 0
    gt = 0
    for wblocks in cfg.windows:
        w = dict(blocks=wblocks, slot0=pos, tile0=gt)
        lo_tiles = []
        hi_tiles = []
        for half_i in (0, 1):
            for b in wblocks:
                seg_start[b, half_i] = pos
                t = int(tiles[b, half_i])
                (lo_tiles if half_i == 0 else hi_tiles).append((b, t))
                pos += t * P
                gt += t
        w["s_lo"] = sum(t for _, t in lo_tiles) * P
        w["s_hi"] = sum(t for _, t in hi_tiles) * P
        w["tiles"] = []
        for b, t in lo_tiles + hi_tiles:
            for _ in range(t):
                w["tiles"].append(b)
        win_meta.append(w)
    tot_slots = pos
    tot_tiles = gt

    # per-block first/last tile bookkeeping (block tiles are split lo/hi and
    # not contiguous; find first and last global tile index per block)
    blk_tiles = [[] for _ in range(bpc)]
    gt = 0
    for w in win_meta:
        for ti, b in enumerate(w["tiles"]):
            blk_tiles[b].append(w["tile0"] + ti)
        gt += len(w["tiles"])

    # per-core slot arrays (vectorized placement)
    out = []
    for c in range(nc_):
        m = core == c
        srcc, blkc, offc, loi = src[m], blk[m], off[m], is_lo[m]
        eidc = np.nonzero(m)[0]
        srcidx = np.zeros(tot_slots, dtype=np.int64)
        uidx = np.zeros(tot_slots, dtype=np.int64)
        offs = np.full(tot_slots, -1.0, dtype=np.float32)
        eslot = np.full(tot_slots, -1, dtype=np.int64)   # edge id per slot
        h = 1 - loi.astype(np.int64)
        order = np.lexsort((srcc, h, blkc))
        gkey = (blkc * 2 + h)[order]
        # rank within each (blk, half) group along the sorted order
        first = np.r_[True, gkey[1:] != gkey[:-1]]
        idxs = np.arange(len(gkey))
        grp_start = idxs[first]
        rank = idxs - np.repeat(grp_start, np.diff(np.r_[grp_start, len(gkey)]))
        s = seg_start[blkc[order], h[order]] + rank
        srcidx[s] = srcc[order] - np.where(h[order] == 1, cfg.half, 0)
        uidx[s] = blkc[order] * P + offc[order]
        offs[s] = offc[order].astype(np.float32)
        eslot[s] = eidc[order]
        out.append(dict(srcidx=srcidx, uidx=uidx, offs=offs, eslot=eslot))

    meta = dict(win=win_meta, tot_slots=tot_slots, tot_tiles=tot_tiles,
                blk_tiles=blk_tiles)
    return out, meta


def wrap16(vals):
    """Wrap a 1-D int index array into the [128, S/16] int16 layout
    (logical position j lives at [j % 16, j // 16], replicated to 128
    partitions for the two descriptor-generating Q7 cores)."""
    n = len(vals)
    S = -(-n // 16)
    flat = np.zeros(16 * S, dtype=np.int16)
    flat[:n] = vals.astype(np.int16)
    arr = np.ascontiguousarray(flat.reshape(S, 16).T)
    return np.tile(arr, (8, 1))


def build_core_inputs(cfg, meta, percore, x_pad):
    """Per-core numpy input dict (device tensor name -> array)."""
    ins = []
    for c in range(cfg.ncores):
        pc = percore[c]
        # index arrays: per window, vw gets lo-seg then hi-seg; u one segment
        vw_cols = []
        u_cols = []
        for w in meta["win"]:
            s0, sl, sh = w["slot0"], w["s_lo"], w["s_hi"]
            sidx = pc["srcidx"][s0:s0 + sl + sh]
            vw_cols.append(wrap16(sidx[:sl]))
            if sh:
                vw_cols.append(wrap16(sidx[sl:]))
            u_cols.append(wrap16(pc["uidx"][s0:s0 + sl + sh]))
        vwidx = np.concatenate(vw_cols, axis=1) if vw_cols else np.zeros((128, 1), np.int16)
        uidxa = np.concatenate(u_cols, axis=1)
        tt = meta["tot_tiles"]
        offs = pc["offs"].reshape(tt, P).T.copy()          # [128, tt]
        x_own = x_pad[c * cfg.nodes_pc:(c + 1) * cfg.nodes_pc]
        xrows = x_own.reshape(cfg.bpc, P, D).transpose(1, 0, 2).reshape(P, cfg.bpc * D)
        ins.append(dict(vwidx=vwidx, uidx=uidxa, offs=offs,
                        xrows=np.ascontiguousarray(xrows, dtype=np.float32)))
    return ins


# ---------------------------------------------------------------- device code
def build_nc(cfg, meta):
    nc = bacc.Bacc("TRN2", target_bir_lowering=False, debug=False,
                   num_devices=cfg.ncores)
    L, bpc, npc = cfg.L, cfg.bpc, cfg.nodes_pc
    tt = meta["tot_tiles"]
    ts = meta["tot_slots"]
    vw_icols = sum(w["s_lo"] // 16 + w["s_hi"] // 16 for w in meta["win"])
    u_icols = sum((w["s_lo"] + w["s_hi"]) // 16 for w in meta["win"])

    # ---------------- I/O
    xrows_d = nc.dram_tensor("xrows", [P, bpc * D], f32, kind="ExternalInput")
    vwidx_d = nc.dram_tensor("vwidx", [P, vw_icols], i16, kind="ExternalInput")
    uidx_d = nc.dram_tensor("uidx", [P, u_icols], i16, kind="ExternalInput")
    offs_d = nc.dram_tensor("offs", [P, tt], f32, kind="ExternalInput")
    ea_d = nc.dram_tensor("ea2", [P, 2 * tt], bf16, kind="ExternalInput")
    wn_d = nc.dram_tensor("wn", [L, D, D], bf16, kind="ExternalInput")
    w1a_d = nc.dram_tensor("w1a", [L, D, H], bf16, kind="ExternalInput")
    w1b_d = nc.dram_tensor("w1b", [L, D, H], bf16, kind="ExternalInput")
    we1_d = nc.dram_tensor("we1", [L, D, D], bf16, kind="ExternalInput")
    we2_d = nc.dram_tensor("we2", [L, 2, D], bf16, kind="ExternalInput")
    w2r_d = nc.dram_tensor("w2r", [L, P, H], bf16, kind="ExternalInput")
    ident_d = nc.dram_tensor("ident", [P, P], f32, kind="ExternalInput")
    iota_d = nc.dram_tensor("iota", [P, P], f32, kind="ExternalInput")
    y_d = nc.dram_tensor("y", [P, bpc * D], f32, kind="ExternalOutput")
    if DEBUG_TAPS:
        dbg_out = nc.dram_tensor("dbg_out", [P, bpc * D], f32, kind="ExternalOutput")
        dbg_att = nc.dram_tensor("dbg_att", [P, tt], f32, kind="ExternalOutput")
        dbg_ug = nc.dram_tensor("dbg_ug", [P, tt * 128], f32, kind="ExternalOutput")
        dbg_vg = nc.dram_tensor("dbg_vg", [P, tt * 256], f32, kind="ExternalOutput")

    # DRAM scratch (double-buffered across layers)
    vw_own = [nc.dram_tensor(f"vw_own{i}", [npc, 256], bf16, kind="Internal")
              for i in range(2)]
    u_own = [nc.dram_tensor(f"u_own{i}", [npc, 128], bf16, kind="Internal")
             for i in range(2)]
    aspace = "Shared" if cfg.ncores > 4 else "Local"
    vw_full = [nc.dram_tensor(f"vw_full{i}", [cfg.npad, 256], bf16,
                              kind="Internal", addr_space=aspace)
               for i in range(2)]

    with tile.TileContext(nc) as tc:
        with (
            tc.tile_pool(name="res", bufs=1) as res,
            tc.tile_pool(name="wp", bufs=2) as wp,
            tc.tile_pool(name="win", bufs=2) as wnp,
            tc.tile_pool(name="satt", bufs=4) as sap,
            tc.tile_pool(name="small", bufs=2) as smp,
            tc.tile_pool(name="accps", bufs=2 * cfg.window_b, space="PSUM") as accps,
            tc.tile_pool(name="scps", bufs=3, space="PSUM") as scps,
        ):
            # ---------------- resident tiles
            xrows = res.tile([P, bpc * D], f32)
            nc.sync.dma_start(xrows[:], xrows_d[:])
            xT16 = res.tile([P, bpc * D], bf16)
            hT16 = res.tile([P, bpc * D], bf16)
            vwidx = res.tile([P, vw_icols], i16)
            nc.sync.dma_start(vwidx[:], vwidx_d[:])
            uidx = res.tile([P, u_icols], i16)
            nc.sync.dma_start(uidx[:], uidx_d[:])
            offs = res.tile([P, tt], f32)
            nc.sync.dma_start(offs[:], offs_d[:])
            ea = res.tile([P, 2 * tt], bf16)
            nc.sync.dma_start(ea[:], ea_d[:])
            ident = res.tile([P, P], f32)
            nc.sync.dma_start(ident[:], ident_d[:])
            iota = res.tile([P, P], f32)
            nc.sync.dma_start(iota[:], iota_d[:])
            eps_sb = res.tile([P, 1], f32)
            nc.vector.memset(eps_sb[:], LN_EPS)

            for l in range(L):
                pb = l % 2
                # ---- layer weights
                wn_sb = wp.tile([D, D], bf16, tag="wn")
                nc.sync.dma_start(wn_sb[:], wn_d[l])
                w1a_sb = wp.tile([D, H], bf16, tag="w1a")
                nc.sync.dma_start(w1a_sb[:], w1a_d[l])
                w1b_sb = wp.tile([D, H], bf16, tag="w1b")
                nc.sync.dma_start(w1b_sb[:], w1b_d[l])
                we1_sb = wp.tile([D, D], bf16, tag="we1")
                nc.sync.dma_start(we1_sb[:], we1_d[l])
                we2_sb = wp.tile([2, D], bf16, tag="we2")
                nc.sync.dma_start(we2_sb[:], we2_d[l])
                w2r_sb = wp.tile([P, H], bf16, tag="w2r")
                nc.sync.dma_start(w2r_sb[:], w2r_d[l])

                # ---- x^T (bf16) for table matmuls
                for b in range(bpc):
                    pt = scps.tile([P, P], f32, space="PSUM", tag="tps")
                    nc.tensor.transpose(pt[:], xrows[:, b * D:(b + 1) * D],
                                        ident[:])
                    nc.scalar.copy(xT16[:, b * D:(b + 1) * D], pt[:])

                # ---- tables: h^T, then u/v/w rows
                for b in range(bpc):
                    sl = slice(b * D, (b + 1) * D)
                    ph = scps.tile([P, P], f32, space="PSUM", tag="tps")
                    nc.tensor.matmul(ph[:], lhsT=wn_sb[:], rhs=xT16[:, sl],
                                     start=True, stop=True)
                    nc.vector.tensor_copy(hT16[:, sl], ph[:])
                    puvw = scps.tile([P, 256], f32, space="PSUM", tag="tps")
                    nc.tensor.matmul(puvw[:, 0:H], lhsT=hT16[:, sl],
                                     rhs=w1a_sb[:], start=True, stop=True)
                    nc.tensor.matmul(puvw[:, H:2 * H], lhsT=hT16[:, sl],
                                     rhs=w1b_sb[:], start=True, stop=True)
                    nc.tensor.matmul(puvw[:, 2 * H:2 * H + D], lhsT=hT16[:, sl],
                                     rhs=we1_sb[:], start=True, stop=True)
                    ust = smp.tile([P, 128], bf16, tag="ust")
                    nc.scalar.copy(ust[:, 0:H], puvw[:, 0:H])
                    nc.vector.memset(ust[:, H:], 0.0)
                    vwst = smp.tile([P, 256], bf16, tag="vwst")
                    nc.vector.memset(vwst[:, H + D:], 0.0)
                    nc.vector.tensor_copy(vwst[:, 0:H], puvw[:, H:2 * H])
                    nc.vector.tensor_copy(vwst[:, H:H + D],
                                          puvw[:, 2 * H:2 * H + D])
                    nc.sync.dma_start(
                        u_own[pb][b * P:(b + 1) * P, :], ust[:])
                    nc.sync.dma_start(
                        vw_own[pb][b * P:(b + 1) * P, :], vwst[:])

                # ---- share the src-side table
                if VARIANT != "tablesonly":
                 nc.gpsimd.collective_compute(
                    "AllGather", AL.bypass,
                    replica_groups=[list(range(cfg.ncores))],
                    ins=[vw_own[pb][:]], outs=[vw_full[pb][:]])

                # ---- edge pass
                vw_col = 0
                u_col = 0
                for w in (meta["win"] if VARIANT not in ("tablesonly", "tabag") else []):
                    wb = w["blocks"]
                    nwb = len(wb)
                    T_w = len(w["tiles"])
                    t_lo = w["s_lo"] // P
                    t_hi = w["s_hi"] // P
                    # gathers
                    vg = wnp.tile([P, T_w, 256], bf16, tag="vg")
                    if VARIANT != "nogather":
                     nc.gpsimd.dma_gather(
                        out_ap=vg[:, 0:t_lo, :], in_ap=vw_full[pb][:],
                        idxs_ap=vwidx[:, vw_col:vw_col + w["s_lo"] // 16],
                        num_idxs=w["s_lo"], num_idxs_reg=w["s_lo"],
                        elem_size=256, single_packet=False)
                    vw_col += w["s_lo"] // 16
                    if t_hi and VARIANT != "nogather":
                        nc.gpsimd.dma_gather(
                            out_ap=vg[:, t_lo:T_w, :],
                            in_ap=vw_full[pb][cfg.half:, :],
                            idxs_ap=vwidx[:, vw_col:vw_col + w["s_hi"] // 16],
                            num_idxs=w["s_hi"], num_idxs_reg=w["s_hi"],
                            elem_size=256, single_packet=False)
                        vw_col += w["s_hi"] // 16
                    elif t_hi:
                        vw_col += w["s_hi"] // 16
                    ug = wnp.tile([P, T_w, 128], bf16, tag="ug")
                    s_all = w["s_lo"] + w["s_hi"]
                    if VARIANT == "nogather":
                        nc.vector.memset(vg[:, 0, 0:8], 0.0)
                        nc.vector.memset(ug[:, 0, 0:8], 0.0)
                    if VARIANT != "nogather":
                     nc.gpsimd.dma_gather(
                        out_ap=ug[:], in_ap=u_own[pb][:],
                        idxs_ap=uidx[:, u_col:u_col + s_all // 16],
                        num_idxs=s_all, num_idxs_reg=s_all, elem_size=128,
                        single_packet=False)
                    u_col += s_all // 16

                    # edge_attr columns into the gathered rows' pad region so
                    # the scatter matmul consumes [w | ea] in one rhs stream.
                    # (reads vg's pad back through in1 to order after the
                    # gathers -- plain WAW on the custom gather is not enough)
                    te0 = w["tile0"]
                    nc.vector.scalar_tensor_tensor(
                        out=vg[:, :, 192:194],
                        in0=ea[:, 2 * te0:2 * (te0 + T_w)]
                        .rearrange("p (t e) -> p t e", e=2),
                        scalar=0.0,
                        in1=vg[:, :, 192:194],
                        op0=AL.add, op1=AL.bypass)

                    # attention
                    if VARIANT == "gonly":
                        continue
                    a1 = wnp.tile([P, T_w, H], bf16, tag="a1")
                    nc.vector.tensor_tensor(
                        out=a1[:], in0=ug[:, :, 0:H], in1=vg[:, :, 0:H],
                        op=AL.add)
                    rw = wnp.tile([P, T_w, H], bf16, tag="rw")
                    w2b = w2r_sb[:].rearrange("p (t e) -> p t e", t=1) \
                                   .broadcast_to((P, T_w, H))
                    nc.vector.scalar_tensor_tensor(
                        out=rw[:], in0=a1[:], scalar=0.0, in1=w2b,
                        op0=AL.max, op1=AL.mult)
                    logit = wnp.tile([P, T_w], f32, tag="logit")
                    nc.vector.tensor_reduce(
                        out=logit[:], in_=rw[:], axis=mybir.AxisListType.X,
                        op=AL.add)
                    att = wnp.tile([P, T_w], f32, tag="att")
                    nc.scalar.activation(att[:], logit[:], AF.Sigmoid)
                    if DEBUG_TAPS and l == 0:
                        t0_ = w["tile0"]
                        nc.sync.dma_start(dbg_att[:, t0_:t0_ + T_w], att[:])
                        nc.gpsimd.dma_start(
                            dbg_ug[:, t0_ * 128:(t0_ + T_w) * 128],
                            ug[:].rearrange("p t e -> p (t e)"))
                        nc.gpsimd.dma_start(
                            dbg_vg[:, t0_ * 256:(t0_ + T_w) * 256],
                            vg[:].rearrange("p t e -> p (t e)"))

                    # scatter: one PSUM bank per block; cols 0:D = segment
                    # sums, cols D:D+2 = P2 (edge_attr factor) -- one group.
                    accs = []
                    for _bi in range(nwb):
                        accb = accps.tile([P, 512], f32, space="PSUM",
                                          tag="accb")
                        accs.append(accb)
                    for ti, b in enumerate(w["tiles"]):
                        gt = w["tile0"] + ti
                        bl = wb.index(b)
                        acc = accs[bl]
                        first = gt == meta["blk_tiles"][b][0]
                        satt = sap.tile([P, P], bf16, tag="satt")
                        nc.vector.tensor_scalar(
                            out=satt[:], in0=iota[:],
                            scalar1=offs[:, gt:gt + 1],
                            scalar2=att[:, ti:ti + 1],
                            op0=AL.is_equal, op1=AL.mult)
                        last = gt == meta["blk_tiles"][b][-1]
                        nc.tensor.matmul(
                            acc[:, 0:D + 2], lhsT=satt[:],
                            rhs=vg[:, ti, H:H + D + 2],
                            start=first, stop=last)

                    # per-block: edge_attr term, then out = acc + P2@We2
                    # materialized in SBUF (PSUM group is closed by now).
                    sqs = smp.tile([P, nwb], f32, tag="sqs")
                    mu = smp.tile([P, nwb], f32, tag="mu")
                    outw = wnp.tile([P, nwb * D], f32, tag="outw")
                    for bl, b in enumerate(wb):
                        acc = accs[bl]
                        p2sb = smp.tile([P, 2], f32, tag="p2sb")
                        nc.scalar.copy(p2sb[:], acc[:, D:D + 2])
                        p2t_ps = scps.tile([P, P], f32, space="PSUM", tag="tps")
                        nc.tensor.transpose(p2t_ps[0:2, :], p2sb[:], ident[:])
                        p2t = smp.tile([2, P], bf16, tag="p2t")
                        nc.scalar.copy(p2t[:], p2t_ps[0:2, :])
                        eat_ps = scps.tile([P, P], f32, space="PSUM", tag="tps")
                        nc.tensor.matmul(eat_ps[:], lhsT=p2t[:], rhs=we2_sb[:],
                                         start=True, stop=True)
                        eat_sb = smp.tile([P, D], f32, tag="eat")
                        nc.scalar.copy(eat_sb[:], eat_ps[:])
                        osl = outw[:, bl * D:(bl + 1) * D]
                        nc.vector.tensor_tensor(out=osl, in0=acc[:, 0:D],
                                                in1=eat_sb[:], op=AL.add)
                        sq_scr = smp.tile([P, D], f32, tag="sqscr")
                        nc.scalar.activation(
                            sq_scr[:], osl, AF.Square,
                            accum_out=sqs[:, bl:bl + 1])
                        nc.vector.tensor_reduce(
                            out=mu[:, bl:bl + 1], in_=osl,
                            axis=mybir.AxisListType.X, op=AL.add)

                    # LayerNorm (+relu) + residual
                    mean = smp.tile([P, nwb], f32, tag="mean")
                    nc.vector.tensor_scalar_mul(mean[:], mu[:], 1.0 / D)
                    m2 = smp.tile([P, nwb], f32, tag="m2")
                    nc.vector.scalar_tensor_tensor(
                        out=m2[:], in0=mu[:], scalar=1.0 / (D * D),
                        in1=mu[:], op0=AL.mult, op1=AL.mult)
                    var = smp.tile([P, nwb], f32, tag="var")
                    nc.vector.scalar_tensor_tensor(
                        out=var[:], in0=sqs[:], scalar=1.0 / D, in1=m2[:],
                        op0=AL.mult, op1=AL.subtract)
                    std = smp.tile([P, nwb], f32, tag="std")
                    nc.scalar.activation(std[:], var[:], AF.Sqrt, bias=eps_sb[:])
                    rstd = smp.tile([P, nwb], f32, tag="rstd")
                    nc.vector.reciprocal(rstd[:], std[:])
                    lnw = wnp.tile([P, nwb * D], f32, tag="lnw")
                    for bl in range(nwb):
                        nc.vector.tensor_scalar(
                            out=lnw[:, bl * D:(bl + 1) * D],
                            in0=outw[:, bl * D:(bl + 1) * D],
                            scalar1=mean[:, bl:bl + 1],
                            scalar2=rstd[:, bl:bl + 1],
                            op0=AL.subtract, op1=AL.mult)
                    if DEBUG_TAPS and l == 0:
                        nc.sync.dma_start(
                            dbg_out[:, wb[0] * D:(wb[0] + nwb) * D], outw[:])
                    if l < L - 1:
                        nc.vector.tensor_scalar_max(lnw[:], lnw[:], 0.0)
                    x_sl = xrows[:, wb[0] * D:(wb[0] + nwb) * D]
                    nc.vector.tensor_tensor(out=x_sl, in0=lnw[:], in1=x_sl,
                                            op=AL.add)

            nc.sync.dma_start(y_d[:], xrows[:])

    nc.compile()
    return nc


# ---------------------------------------------------------------- entry point
def make_in_maps(cfg, meta, percore, core_in, inputs):
    edge_attr = np.asarray(inputs["edge_attr"], dtype=np.float32)
    lin_node_w = np.asarray(inputs["lin_node_w"], dtype=np.float32)
    lin_edge_w = np.asarray(inputs["lin_edge_w"], dtype=np.float32)
    att_w1 = np.asarray(inputs["att_w1"], dtype=np.float32)
    att_w2 = np.asarray(inputs["att_w2"], dtype=np.float32)
    L = cfg.L
    wn = lin_node_w.astype(BF)
    w1a = att_w1[:, :D, :].astype(BF)
    w1b = att_w1[:, D:, :].astype(BF)
    we1 = lin_edge_w[:, :D, :].astype(BF)
    we2 = lin_edge_w[:, D:, :].astype(BF)
    w2r = np.broadcast_to(att_w2[:, :, 0][:, None, :], (L, P, H)).astype(BF)
    ident = np.eye(P, dtype=np.float32)
    iota = np.broadcast_to(np.arange(P, dtype=np.float32), (P, P)).copy()
    tt = meta["tot_tiles"]
    in_maps = []
    for c in range(cfg.ncores):
        ea_slots = np.zeros((tt * P, 2), dtype=np.float32)
        valid = percore[c]["eslot"] >= 0
        ea_slots[valid] = edge_attr[percore[c]["eslot"][valid]]
        ea2 = ea_slots.reshape(tt, P, 2).transpose(1, 0, 2).reshape(P, 2 * tt)
        in_maps.append(dict(
            xrows=core_in[c]["xrows"],
            vwidx=core_in[c]["vwidx"].astype(np.int16),
            uidx=core_in[c]["uidx"].astype(np.int16),
            offs=core_in[c]["offs"].astype(np.float32),
            ea2=np.ascontiguousarray(ea2).astype(BF),
            wn=wn, w1a=w1a, w1b=w1b, we1=we1, we2=we2, w2r=w2r,
            ident=ident, iota=iota,
        ))
    return in_maps



def run(cfg, inputs, nc=None):
    x = np.asarray(inputs["x"], dtype=np.float32)
    edge_index = np.asarray(inputs["edge_index"])
    edge_attr = np.asarray(inputs["edge_attr"], dtype=np.float32)
    lin_node_w = np.asarray(inputs["lin_node_w"], dtype=np.float32)
    lin_edge_w = np.asarray(inputs["lin_edge_w"], dtype=np.float32)
    att_w1 = np.asarray(inputs["att_w1"], dtype=np.float32)
    att_w2 = np.asarray(inputs["att_w2"], dtype=np.float32)

    for name in ("att_b1", "att_b2", "bias", "ln_beta"):
        assert not np.any(np.asarray(inputs[name])), f"{name} must be zero"
    assert np.all(np.asarray(inputs["ln_gamma"]) == 1.0), "ln_gamma must be 1"

    percore, meta = prep_edges(cfg, edge_index)
    x_pad = np.zeros((cfg.npad, D), dtype=np.float32)
    x_pad[:cfg.N] = x
    core_in = build_core_inputs(cfg, meta, percore, x_pad)

    in_maps = make_in_maps(cfg, meta, percore, core_in, inputs)

    if nc is None:
        nc = build_nc(cfg, meta)
    res = bass_utils.run_bass_kernel_spmd(
        nc, in_maps, core_ids=list(range(cfg.ncores)))
    outs = []
    for c in range(cfg.ncores):
        yr = res.results[c]["y"]                     # [128, bpc*128]
        outs.append(yr.reshape(P, cfg.bpc, D).transpose(1, 0, 2)
                    .reshape(cfg.nodes_pc, D))
    full = np.concatenate(outs, axis=0)[:cfg.N]
    return np.ascontiguousarray(full), nc, in_maps, meta


def kernel(**inputs) -> np.ndarray:
    out, _, _, _ = run(REAL, inputs)
    return out



# revision 12
# speedup vs baseline: 1.1736x; 1.1736x over previous
"""AdmittanceGNN (3-layer edge-attention GNN) on 8 Trainium2 NeuronCores.

Strategy (dst-sharded):
  - Nodes are sharded into 8 contiguous ranges (6272/core, padded to 50176).
  - Each core owns all edges whose dst falls in its range -> segment sums are
    core-local (no big cross-core reduction).
  - Per layer, each core computes feature tables for ITS nodes:
      u = (x@Wn)@W1a   (dst-side attention term, stays local)
      v = (x@Wn)@W1b   (src-side attention term)
      w = (x@Wn)@We1   (src-side message term)
    [v|w] rows are AllGathered into a full table; per-edge v/w are fetched with
    batched dma_gather (int16 indices, windowed by table halves).
  - Attention: a1 = relu(u[dst]+v[src]); att = sigmoid(sum(a1*w2)).
  - Messages never materialize att*(...) per edge: att is folded into a
    one-hot scatter matrix S_att[e, n] = (n == dst_off(e)) * att(e); segment
    sum = S_att^T @ w_gathered via PE matmuls accumulating in PSUM.
    The edge_attr term factors through k=2: P2 = S_att^T @ ea (per block),
    then block += P2 @ We2.
  - LayerNorm + relu + residual per 128-node block, fp32 residual stream.
"""
import math

import numpy as np
import ml_dtypes

import concourse.bass as bass
import concourse.bacc as bacc
import concourse.tile as tile
import concourse.mybir as mybir
from concourse import bass_utils

P = 128
D = 128
H = 64
LN_EPS = 1e-5

f32 = mybir.dt.float32
bf16 = mybir.dt.bfloat16
i16 = mybir.dt.int16
BF = ml_dtypes.bfloat16

DEBUG_TAPS = False
import os
VARIANT = os.environ.get("KVARIANT", "full")  # full | gonly | nogather | notables

AL = mybir.AluOpType
AF = mybir.ActivationFunctionType


class Cfg:
    def __init__(self, N, E, L, ncores, bpc, window_b=2, half=32768):
        self.N, self.E, self.L, self.ncores = N, E, L, ncores
        self.bpc = bpc                      # blocks of 128 nodes per core
        self.nodes_pc = bpc * P
        self.npad = ncores * self.nodes_pc
        self.window_b = window_b
        self.half = half                    # src-index window split
        assert self.npad >= N
        # windows: list of lists of block indices
        self.windows = [list(range(i, min(i + window_b, bpc)))
                        for i in range(0, bpc, window_b)]


REAL = Cfg(N=50000, E=640000, L=3, ncores=8, bpc=49, window_b=2, half=32768)


# ---------------------------------------------------------------- host prep
def prep_edges(cfg, edge_index):
    """Bucket/sort/pad edges; build per-core slot arrays + shared layout.

    Slot order (identical across cores): for each window w (window_b blocks):
      [lo-seg(b0) | lo-seg(b1) | ... | hi-seg(b0) | hi-seg(b1) | ...]
    each segment padded to a multiple of 128. Slot s -> tile t=s//128,
    partition p=s%128.
    """
    src = np.asarray(edge_index[0], dtype=np.int64)
    dst = np.asarray(edge_index[1], dtype=np.int64)
    E = len(src)
    nc_, bpc, npc = cfg.ncores, cfg.bpc, cfg.nodes_pc

    core = dst // npc
    loc = dst - core * npc
    blk = loc // P
    off = loc % P
    is_lo = src < cfg.half

    # counts per (core, blk, half)
    cnt = np.zeros((nc_, bpc, 2), dtype=np.int64)
    np.add.at(cnt, (core, blk, 1 - is_lo.astype(np.int64)), 1)
    tiles = np.maximum(0, -(-cnt.max(axis=0) // P))        # [bpc, 2] shared
    seg_slots = tiles * P

    # global layout
    seg_start = np.zeros((bpc, 2), dtype=np.int64)
    tile_block = []     # per global tile: block index
    tile_first = []     # is first tile of its block (PSUM start)
    tile_last = []      # is last tile of its block (PSUM stop)
    win_meta = []       # per window: dict
    pos = 0
    gt = 0
    for wblocks in cfg.windows:
        w = dict(blocks=wblocks, slot0=pos, tile0=gt)
        lo_tiles = []
        hi_tiles = []
        for half_i in (0, 1):
            for b in wblocks:
                seg_start[b, half_i] = pos
                t = int(tiles[b, half_i])
                (lo_tiles if half_i == 0 else hi_tiles).append((b, t))
                pos += t * P
                gt += t
        w["s_lo"] = sum(t for _, t in lo_tiles) * P
        w["s_hi"] = sum(t for _, t in hi_tiles) * P
        w["tiles"] = []
        for b, t in lo_tiles + hi_tiles:
            for _ in range(t):
                w["tiles"].append(b)
        win_meta.append(w)
    tot_slots = pos
    tot_tiles = gt

    # per-block first/last tile bookkeeping (block tiles are split lo/hi and
    # not contiguous; find first and last global tile index per block)
    blk_tiles = [[] for _ in range(bpc)]
    gt = 0
    for w in win_meta:
        for ti, b in enumerate(w["tiles"]):
            blk_tiles[b].append(w["tile0"] + ti)
        gt += len(w["tiles"])

    # per-core slot arrays (vectorized placement)
    out = []
    for c in range(nc_):
        m = core == c
        srcc, blkc, offc, loi = src[m], blk[m], off[m], is_lo[m]
        eidc = np.nonzero(m)[0]
        srcidx = np.zeros(tot_slots, dtype=np.int64)
        uidx = np.zeros(tot_slots, dtype=np.int64)
        offs = np.full(tot_slots, -1.0, dtype=np.float32)
        eslot = np.full(tot_slots, -1, dtype=np.int64)   # edge id per slot
        h = 1 - loi.astype(np.int64)
        order = np.lexsort((srcc, h, blkc))
        gkey = (blkc * 2 + h)[order]
        # rank within each (blk, half) group along the sorted order
        first = np.r_[True, gkey[1:] != gkey[:-1]]
        idxs = np.arange(len(gkey))
        grp_start = idxs[first]
        rank = idxs - np.repeat(grp_start, np.diff(np.r_[grp_start, len(gkey)]))
        s = seg_start[blkc[order], h[order]] + rank
        srcidx[s] = srcc[order] - np.where(h[order] == 1, cfg.half, 0)
        uidx[s] = blkc[order] * P + offc[order]
        offs[s] = offc[order].astype(np.float32)
        eslot[s] = eidc[order]
        out.append(dict(srcidx=srcidx, uidx=uidx, offs=offs, eslot=eslot))

    meta = dict(win=win_meta, tot_slots=tot_slots, tot_tiles=tot_tiles,
                blk_tiles=blk_tiles)
    return out, meta


def wrap16(vals):
    """Wrap a 1-D int index array into the [128, S/16] int16 layout
    (logical position j lives at [j % 16, j // 16], replicated to 128
    partitions for the two descriptor-generating Q7 cores)."""
    n = len(vals)
    S = -(-n // 16)
    flat = np.zeros(16 * S, dtype=np.int16)
    flat[:n] = vals.astype(np.int16)
    arr = np.ascontiguousarray(flat.reshape(S, 16).T)
    return np.tile(arr, (8, 1))


def build_core_inputs(cfg, meta, percore, x_pad):
    """Per-core numpy input dict (device tensor name -> array)."""
    ins = []
    for c in range(cfg.ncores):
        pc = percore[c]
        # index arrays: per window, vw gets lo-seg then hi-seg; u one segment
        vw_cols = []
        u_cols = []
        for w in meta["win"]:
            s0, sl, sh = w["slot0"], w["s_lo"], w["s_hi"]
            sidx = pc["srcidx"][s0:s0 + sl + sh]
            vw_cols.append(wrap16(sidx[:sl]))
            if sh:
                vw_cols.append(wrap16(sidx[sl:]))
            u_cols.append(wrap16(pc["uidx"][s0:s0 + sl + sh]))
        vwidx = np.concatenate(vw_cols, axis=1) if vw_cols else np.zeros((128, 1), np.int16)
        uidxa = np.concatenate(u_cols, axis=1)
        tt = meta["tot_tiles"]
        offs = pc["offs"].reshape(tt, P).T.copy()          # [128, tt]
        x_own = x_pad[c * cfg.nodes_pc:(c + 1) * cfg.nodes_pc]
        xrows = x_own.reshape(cfg.bpc, P, D).transpose(1, 0, 2).reshape(P, cfg.bpc * D)
        ins.append(dict(vwidx=vwidx, uidx=uidxa, offs=offs,
                        xrows=np.ascontiguousarray(xrows, dtype=np.float32)))
    return ins


# ---------------------------------------------------------------- device code
def build_nc(cfg, meta):
    nc = bacc.Bacc("TRN2", target_bir_lowering=False, debug=False,
                   num_devices=cfg.ncores, num_swdge_queues=2)
    L, bpc, npc = cfg.L, cfg.bpc, cfg.nodes_pc
    tt = meta["tot_tiles"]
    ts = meta["tot_slots"]
    vw_icols = sum(w["s_lo"] // 16 + w["s_hi"] // 16 for w in meta["win"])
    u_icols = sum((w["s_lo"] + w["s_hi"]) // 16 for w in meta["win"])

    # ---------------- I/O
    xrows_d = nc.dram_tensor("xrows", [P, bpc * D], f32, kind="ExternalInput")
    vwidx_d = nc.dram_tensor("vwidx", [P, vw_icols], i16, kind="ExternalInput")
    uidx_d = nc.dram_tensor("uidx", [P, u_icols], i16, kind="ExternalInput")
    offs_d = nc.dram_tensor("offs", [P, tt], f32, kind="ExternalInput")
    ea_d = nc.dram_tensor("ea2", [P, 2 * tt], bf16, kind="ExternalInput")
    wn_d = nc.dram_tensor("wn", [L, D, D], bf16, kind="ExternalInput")
    w1a_d = nc.dram_tensor("w1a", [L, D, H], bf16, kind="ExternalInput")
    w1b_d = nc.dram_tensor("w1b", [L, D, H], bf16, kind="ExternalInput")
    we1_d = nc.dram_tensor("we1", [L, D, D], bf16, kind="ExternalInput")
    we2_d = nc.dram_tensor("we2", [L, 2, D], bf16, kind="ExternalInput")
    w2r_d = nc.dram_tensor("w2r", [L, P, H], bf16, kind="ExternalInput")
    ident_d = nc.dram_tensor("ident", [P, P], f32, kind="ExternalInput")
    iota_d = nc.dram_tensor("iota", [P, P], bf16, kind="ExternalInput")
    y_d = nc.dram_tensor("y", [P, bpc * D], f32, kind="ExternalOutput")
    if DEBUG_TAPS:
        dbg_out = nc.dram_tensor("dbg_out", [P, bpc * D], f32, kind="ExternalOutput")
        dbg_att = nc.dram_tensor("dbg_att", [P, tt], f32, kind="ExternalOutput")
        dbg_ug = nc.dram_tensor("dbg_ug", [P, tt * 128], f32, kind="ExternalOutput")
        dbg_vg = nc.dram_tensor("dbg_vg", [P, tt * 256], f32, kind="ExternalOutput")

    # DRAM scratch (double-buffered across layers)
    vw_own = [nc.dram_tensor(f"vw_own{i}", [npc, 256], bf16, kind="Internal")
              for i in range(2)]
    u_own = [nc.dram_tensor(f"u_own{i}", [npc, 128], bf16, kind="Internal")
             for i in range(2)]
    aspace = "Shared" if cfg.ncores > 4 else "Local"
    vw_full = [nc.dram_tensor(f"vw_full{i}", [cfg.npad, 256], bf16,
                              kind="Internal", addr_space=aspace)
               for i in range(2)]

    with tile.TileContext(nc) as tc:
        with (
            tc.tile_pool(name="res", bufs=1) as res,
            tc.tile_pool(name="wp", bufs=2) as wp,
            tc.tile_pool(name="win", bufs=2) as wnp,
            tc.tile_pool(name="satt", bufs=4) as sap,
            tc.tile_pool(name="small", bufs=2) as smp,
            tc.tile_pool(name="accps", bufs=2 * cfg.window_b, space="PSUM") as accps,
            tc.tile_pool(name="scps", bufs=3, space="PSUM") as scps,
        ):
            # ---------------- resident tiles
            xrows = res.tile([P, bpc * D], f32)
            nc.sync.dma_start(xrows[:], xrows_d[:])
            xT16 = res.tile([P, bpc * D], bf16)
            hT16 = res.tile([P, bpc * D], bf16)
            vwidx = res.tile([P, vw_icols], i16)
            nc.sync.dma_start(vwidx[:], vwidx_d[:])
            uidx = res.tile([P, u_icols], i16)
            nc.sync.dma_start(uidx[:], uidx_d[:])
            offs = res.tile([P, tt], f32)
            nc.sync.dma_start(offs[:], offs_d[:])
            ea = res.tile([P, 2 * tt], bf16)
            nc.sync.dma_start(ea[:], ea_d[:])
            ident = res.tile([P, P], f32)
            nc.sync.dma_start(ident[:], ident_d[:])
            iota = res.tile([P, P], bf16)
            nc.sync.dma_start(iota[:], iota_d[:])
            eps_sb = res.tile([P, 1], f32)
            nc.vector.memset(eps_sb[:], LN_EPS)

            for l in range(L):
                pb = l % 2
                # ---- layer weights
                wn_sb = wp.tile([D, D], bf16, tag="wn")
                nc.sync.dma_start(wn_sb[:], wn_d[l])
                w1a_sb = wp.tile([D, H], bf16, tag="w1a")
                nc.sync.dma_start(w1a_sb[:], w1a_d[l])
                w1b_sb = wp.tile([D, H], bf16, tag="w1b")
                nc.sync.dma_start(w1b_sb[:], w1b_d[l])
                we1_sb = wp.tile([D, D], bf16, tag="we1")
                nc.sync.dma_start(we1_sb[:], we1_d[l])
                we2_sb = wp.tile([2, D], bf16, tag="we2")
                nc.sync.dma_start(we2_sb[:], we2_d[l])
                w2r_sb = wp.tile([P, H], bf16, tag="w2r")
                nc.sync.dma_start(w2r_sb[:], w2r_d[l])

                # ---- x^T (bf16) for table matmuls
                for b in range(bpc):
                    pt = scps.tile([P, P], f32, space="PSUM", tag="tps")
                    nc.tensor.transpose(pt[:], xrows[:, b * D:(b + 1) * D],
                                        ident[:])
                    nc.scalar.copy(xT16[:, b * D:(b + 1) * D], pt[:])

                # ---- tables: h^T, then u/v/w rows
                for b in range(bpc):
                    sl = slice(b * D, (b + 1) * D)
                    ph = scps.tile([P, P], f32, space="PSUM", tag="tps")
                    nc.tensor.matmul(ph[:], lhsT=wn_sb[:], rhs=xT16[:, sl],
                                     start=True, stop=True)
                    nc.vector.tensor_copy(hT16[:, sl], ph[:])
                    puvw = scps.tile([P, 256], f32, space="PSUM", tag="tps")
                    nc.tensor.matmul(puvw[:, 0:H], lhsT=hT16[:, sl],
                                     rhs=w1a_sb[:], start=True, stop=True)
                    nc.tensor.matmul(puvw[:, H:2 * H], lhsT=hT16[:, sl],
                                     rhs=w1b_sb[:], start=True, stop=True)
                    nc.tensor.matmul(puvw[:, 2 * H:2 * H + D], lhsT=hT16[:, sl],
                                     rhs=we1_sb[:], start=True, stop=True)
                    ust = smp.tile([P, 128], bf16, tag="ust")
                    nc.scalar.copy(ust[:, 0:H], puvw[:, 0:H])
                    nc.vector.memset(ust[:, H:], 0.0)
                    vwst = smp.tile([P, 256], bf16, tag="vwst")
                    nc.vector.memset(vwst[:, H + D:], 0.0)
                    nc.vector.tensor_copy(vwst[:, 0:H], puvw[:, H:2 * H])
                    nc.vector.tensor_copy(vwst[:, H:H + D],
                                          puvw[:, 2 * H:2 * H + D])
                    nc.sync.dma_start(
                        u_own[pb][b * P:(b + 1) * P, :], ust[:])
                    nc.sync.dma_start(
                        vw_own[pb][b * P:(b + 1) * P, :], vwst[:])

                # ---- share the src-side table
                if VARIANT != "tablesonly":
                 nc.gpsimd.collective_compute(
                    "AllGather", AL.bypass,
                    replica_groups=[list(range(cfg.ncores))],
                    ins=[vw_own[pb][:]], outs=[vw_full[pb][:]])

                # ---- edge pass
                vw_col = 0
                u_col = 0
                for w in (meta["win"] if VARIANT not in ("tablesonly", "tabag") else []):
                    wb = w["blocks"]
                    nwb = len(wb)
                    T_w = len(w["tiles"])
                    t_lo = w["s_lo"] // P
                    t_hi = w["s_hi"] // P
                    # gathers
                    vg = wnp.tile([P, T_w, 256], bf16, tag="vg")
                    if VARIANT != "nogather":
                     nc.gpsimd.dma_gather(
                        out_ap=vg[:, 0:t_lo, :], in_ap=vw_full[pb][:],
                        idxs_ap=vwidx[:, vw_col:vw_col + w["s_lo"] // 16],
                        num_idxs=w["s_lo"], num_idxs_reg=w["s_lo"],
                        elem_size=256, single_packet=False, queue_num=0)
                    vw_col += w["s_lo"] // 16
                    if t_hi and VARIANT != "nogather":
                        nc.gpsimd.dma_gather(
                            out_ap=vg[:, t_lo:T_w, :],
                            in_ap=vw_full[pb][cfg.half:, :],
                            idxs_ap=vwidx[:, vw_col:vw_col + w["s_hi"] // 16],
                            num_idxs=w["s_hi"], num_idxs_reg=w["s_hi"],
                            elem_size=256, single_packet=False, queue_num=0)
                        vw_col += w["s_hi"] // 16
                    elif t_hi:
                        vw_col += w["s_hi"] // 16
                    ug = wnp.tile([P, T_w, 128], bf16, tag="ug")
                    s_all = w["s_lo"] + w["s_hi"]
                    if VARIANT == "nogather":
                        nc.vector.memset(vg[:, 0, 0:8], 0.0)
                        nc.vector.memset(ug[:, 0, 0:8], 0.0)
                    if VARIANT != "nogather":
                     nc.gpsimd.dma_gather(
                        out_ap=ug[:], in_ap=u_own[pb][:],
                        idxs_ap=uidx[:, u_col:u_col + s_all // 16],
                        num_idxs=s_all, num_idxs_reg=s_all, elem_size=128,
                        single_packet=False, queue_num=1)
                    u_col += s_all // 16

                    # edge_attr columns into the gathered rows' pad region so
                    # the scatter matmul consumes [w | ea] in one rhs stream.
                    # (reads vg's pad back through in1 to order after the
                    # gathers -- plain WAW on the custom gather is not enough)
                    te0 = w["tile0"]
                    nc.vector.scalar_tensor_tensor(
                        out=vg[:, :, 192:194],
                        in0=ea[:, 2 * te0:2 * (te0 + T_w)]
                        .rearrange("p (t e) -> p t e", e=2),
                        scalar=0.0,
                        in1=vg[:, :, 192:194],
                        op0=AL.add, op1=AL.bypass)

                    # attention
                    if VARIANT == "gonly":
                        continue
                    a1 = wnp.tile([P, T_w, H], bf16, tag="a1")
                    nc.vector.tensor_tensor(
                        out=a1[:], in0=ug[:, :, 0:H], in1=vg[:, :, 0:H],
                        op=AL.add)
                    rw = wnp.tile([P, T_w, H], bf16, tag="rw")
                    w2b = w2r_sb[:].rearrange("p (t e) -> p t e", t=1) \
                                   .broadcast_to((P, T_w, H))
                    nc.vector.scalar_tensor_tensor(
                        out=rw[:], in0=a1[:], scalar=0.0, in1=w2b,
                        op0=AL.max, op1=AL.mult)
                    logit = wnp.tile([P, T_w], f32, tag="logit")
                    nc.vector.tensor_reduce(
                        out=logit[:], in_=rw[:], axis=mybir.AxisListType.X,
                        op=AL.add)
                    att = wnp.tile([P, T_w], f32, tag="att")
                    nc.scalar.activation(att[:], logit[:], AF.Sigmoid)
                    if DEBUG_TAPS and l == 0:
                        t0_ = w["tile0"]
                        nc.sync.dma_start(dbg_att[:, t0_:t0_ + T_w], att[:])
                        nc.gpsimd.dma_start(
                            dbg_ug[:, t0_ * 128:(t0_ + T_w) * 128],
                            ug[:].rearrange("p t e -> p (t e)"))
                        nc.gpsimd.dma_start(
                            dbg_vg[:, t0_ * 256:(t0_ + T_w) * 256],
                            vg[:].rearrange("p t e -> p (t e)"))

                    # scatter: one PSUM bank per block; cols 0:D = segment
                    # sums, cols D:D+2 = P2 (edge_attr factor) -- one group.
                    accs = []
                    for _bi in range(nwb):
                        accb = accps.tile([P, 512], f32, space="PSUM",
                                          tag="accb")
                        accs.append(accb)
                    for ti, b in enumerate(w["tiles"]):
                        gt = w["tile0"] + ti
                        bl = wb.index(b)
                        acc = accs[bl]
                        first = gt == meta["blk_tiles"][b][0]
                        satt = sap.tile([P, P], bf16, tag="satt")
                        nc.vector.tensor_scalar(
                            out=satt[:], in0=iota[:],
                            scalar1=offs[:, gt:gt + 1],
                            scalar2=att[:, ti:ti + 1],
                            op0=AL.is_equal, op1=AL.mult)
                        last = gt == meta["blk_tiles"][b][-1]
                        nc.tensor.matmul(
                            acc[:, 0:D + 2], lhsT=satt[:],
                            rhs=vg[:, ti, H:H + D + 2],
                            start=first, stop=last)

                    # per-block: edge_attr term, then out = acc + P2@We2
                    # materialized in SBUF (PSUM group is closed by now).
                    sqs = smp.tile([P, nwb], f32, tag="sqs")
                    mu = smp.tile([P, nwb], f32, tag="mu")
                    outw = wnp.tile([P, nwb * D], f32, tag="outw")
                    for bl, b in enumerate(wb):
                        acc = accs[bl]
                        p2sb = smp.tile([P, 2], f32, tag="p2sb")
                        nc.scalar.copy(p2sb[:], acc[:, D:D + 2])
                        p2t_ps = scps.tile([P, P], f32, space="PSUM", tag="tps")
                        nc.tensor.transpose(p2t_ps[0:2, :], p2sb[:], ident[:])
                        p2t = smp.tile([2, P], bf16, tag="p2t")
                        nc.scalar.copy(p2t[:], p2t_ps[0:2, :])
                        eat_ps = scps.tile([P, P], f32, space="PSUM", tag="tps")
                        nc.tensor.matmul(eat_ps[:], lhsT=p2t[:], rhs=we2_sb[:],
                                         start=True, stop=True)
                        eat_sb = smp.tile([P, D], f32, tag="eat")
                        nc.scalar.copy(eat_sb[:], eat_ps[:])
                        osl = outw[:, bl * D:(bl + 1) * D]
                        nc.vector.tensor_tensor(out=osl, in0=acc[:, 0:D],
                                                in1=eat_sb[:], op=AL.add)
                        sq_scr = smp.tile([P, D], f32, tag="sqscr")
                        nc.scalar.activation(
                            sq_scr[:], osl, AF.Square,
                            accum_out=sqs[:, bl:bl + 1])
                        nc.vector.tensor_reduce(
                            out=mu[:, bl:bl + 1], in_=osl,
                            axis=mybir.AxisListType.X, op=AL.add)

                    # LayerNorm (+relu) + residual
                    mean = smp.tile([P, nwb], f32, tag="mean")
                    nc.vector.tensor_scalar_mul(mean[:], mu[:], 1.0 / D)
                    m2 = smp.tile([P, nwb], f32, tag="m2")
                    nc.vector.scalar_tensor_tensor(
                        out=m2[:], in0=mu[:], scalar=1.0 / (D * D),
                        in1=mu[:], op0=AL.mult, op1=AL.mult)
                    var = smp.tile([P, nwb], f32, tag="var")
                    nc.vector.scalar_tensor_tensor(
                        out=var[:], in0=sqs[:], scalar=1.0 / D, in1=m2[:],
                        op0=AL.mult, op1=AL.subtract)
                    std = smp.tile([P, nwb], f32, tag="std")
                    nc.scalar.activation(std[:], var[:], AF.Sqrt, bias=eps_sb[:])
                    rstd = smp.tile([P, nwb], f32, tag="rstd")
                    nc.vector.reciprocal(rstd[:], std[:])
                    lnw = wnp.tile([P, nwb * D], f32, tag="lnw")
                    for bl in range(nwb):
                        nc.vector.tensor_scalar(
                            out=lnw[:, bl * D:(bl + 1) * D],
                            in0=outw[:, bl * D:(bl + 1) * D],
                            scalar1=mean[:, bl:bl + 1],
                            scalar2=rstd[:, bl:bl + 1],
                            op0=AL.subtract, op1=AL.mult)
                    if DEBUG_TAPS and l == 0:
                        nc.sync.dma_start(
                            dbg_out[:, wb[0] * D:(wb[0] + nwb) * D], outw[:])
                    if l < L - 1:
                        nc.vector.tensor_scalar_max(lnw[:], lnw[:], 0.0)
                    x_sl = xrows[:, wb[0] * D:(wb[0] + nwb) * D]
                    nc.vector.tensor_tensor(out=x_sl, in0=lnw[:], in1=x_sl,
                                            op=AL.add)

            nc.sync.dma_start(y_d[:], xrows[:])

    nc.compile()
    return nc


# ---------------------------------------------------------------- entry point
def make_in_maps(cfg, meta, percore, core_in, inputs):
    edge_attr = np.asarray(inputs["edge_attr"], dtype=np.float32)
    lin_node_w = np.asarray(inputs["lin_node_w"], dtype=np.float32)
    lin_edge_w = np.asarray(inputs["lin_edge_w"], dtype=np.float32)
    att_w1 = np.asarray(inputs["att_w1"], dtype=np.float32)
    att_w2 = np.asarray(inputs["att_w2"], dtype=np.float32)
    L = cfg.L
    wn = lin_node_w.astype(BF)
    w1a = att_w1[:, :D, :].astype(BF)
    w1b = att_w1[:, D:, :].astype(BF)
    we1 = lin_edge_w[:, :D, :].astype(BF)
    we2 = lin_edge_w[:, D:, :].astype(BF)
    w2r = np.broadcast_to(att_w2[:, :, 0][:, None, :], (L, P, H)).astype(BF)
    ident = np.eye(P, dtype=np.float32)
    iota = np.broadcast_to(np.arange(P, dtype=np.float32), (P, P)).astype(BF)
    tt = meta["tot_tiles"]
    in_maps = []
    for c in range(cfg.ncores):
        ea_slots = np.zeros((tt * P, 2), dtype=np.float32)
        valid = percore[c]["eslot"] >= 0
        ea_slots[valid] = edge_attr[percore[c]["eslot"][valid]]
        ea2 = ea_slots.reshape(tt, P, 2).transpose(1, 0, 2).reshape(P, 2 * tt)
        in_maps.append(dict(
            xrows=core_in[c]["xrows"],
            vwidx=core_in[c]["vwidx"].astype(np.int16),
            uidx=core_in[c]["uidx"].astype(np.int16),
            offs=core_in[c]["offs"].astype(np.float32),
            ea2=np.ascontiguousarray(ea2).astype(BF),
            wn=wn, w1a=w1a, w1b=w1b, we1=we1, we2=we2, w2r=w2r,
            ident=ident, iota=iota,
        ))
    return in_maps



def run(cfg, inputs, nc=None):
    x = np.asarray(inputs["x"], dtype=np.float32)
    edge_index = np.asarray(inputs["edge_index"])
    edge_attr = np.asarray(inputs["edge_attr"], dtype=np.float32)
    lin_node_w = np.asarray(inputs["lin_node_w"], dtype=np.float32)
    lin_edge_w = np.asarray(inputs["lin_edge_w"], dtype=np.float32)
    att_w1 = np.asarray(inputs["att_w1"], dtype=np.float32)
    att_w2 = np.asarray(inputs["att_w2"], dtype=np.float32)

    for name in ("att_b1", "att_b2", "bias", "ln_beta"):
        assert not np.any(np.asarray(inputs[name])), f"{name} must be zero"
    assert np.all(np.asarray(inputs["ln_gamma"]) == 1.0), "ln_gamma must be 1"

    percore, meta = prep_edges(cfg, edge_index)
    x_pad = np.zeros((cfg.npad, D), dtype=np.float32)
    x_pad[:cfg.N] = x
    core_in = build_core_inputs(cfg, meta, percore, x_pad)

    in_maps = make_in_maps(cfg, meta, percore, core_in, inputs)

    if nc is None:
        nc = build_nc(cfg, meta)
    res = bass_utils.run_bass_kernel_spmd(
        nc, in_maps, core_ids=list(range(cfg.ncores)))
    outs = []
    for c in range(cfg.ncores):
        yr = res.results[c]["y"]                     # [128, bpc*128]
        outs.append(yr.reshape(P, cfg.bpc, D).transpose(1, 0, 2)
                    .reshape(cfg.nodes_pc, D))
    full = np.concatenate(outs, axis=0)[:cfg.N]
    return np.ascontiguousarray(full), nc, in_maps, meta


_CACHE: dict = {}


def _make_runner(nc, ncores):
    """Persistent sharded PJRT runner (compile once, reuse across calls)."""
    import jax
    from jax.sharding import Mesh, PartitionSpec, NamedSharding
    from jax.experimental.shard_map import shard_map
    from concourse import bass2jax

    bass2jax.install_neuronx_cc_hook()
    partition_name = (nc.partition_id_tensor.name
                      if nc.partition_id_tensor else None)
    in_names, out_names, out_avals, zero_shapes = [], [], [], []
    for alloc in nc.m.functions[0].allocations:
        if not isinstance(alloc, mybir.MemoryLocationSet):
            continue
        name = alloc.memorylocations[0].name
        if alloc.kind == "ExternalInput":
            if name != partition_name:
                in_names.append(name)
        elif alloc.kind == "ExternalOutput":
            out_names.append(name)
            shape = tuple(alloc.tensor_shape)
            dtype = mybir.dt.np(alloc.dtype)
            out_avals.append(jax.core.ShapedArray(shape, dtype))
            zero_shapes.append((shape, dtype))
    n_params = len(in_names)
    all_in = in_names + out_names + ([partition_name] if partition_name else [])

    def _body(*args):
        operands = list(args)
        if partition_name is not None:
            operands.append(bass2jax.partition_id_tensor())
        outs = bass2jax._bass_exec_p.bind(
            *operands, out_avals=tuple(out_avals), in_names=tuple(all_in),
            out_names=tuple(out_names), lowering_input_output_aliases=(),
            sim_require_finite=True, sim_require_nnan=True, nc=nc)
        return tuple(outs)

    devices = jax.devices()[:ncores]
    mesh = Mesh(np.asarray(devices), ("core",))
    nz = len(zero_shapes)
    sharded = jax.jit(
        shard_map(_body, mesh=mesh,
                  in_specs=(PartitionSpec("core"),) * (n_params + nz),
                  out_specs=(PartitionSpec("core"),) * len(out_names),
                  check_rep=False),
        keep_unused=True)
    sharding = NamedSharding(mesh, PartitionSpec("core"))

    def call(in_maps):
        import jax as _jax
        concat = [np.concatenate([np.asarray(m[name]) for m in in_maps],
                                 axis=0) for name in in_names]
        zeros = [np.zeros((ncores * s[0], *s[1:]), d)
                 for (s, d) in zero_shapes]
        dev = [_jax.device_put(a, sharding) for a in concat + zeros]
        outs = sharded(*dev)
        _jax.block_until_ready(outs)
        return [
            {name: np.asarray(outs[i]).reshape(ncores, *out_avals[i].shape)[c]
             for i, name in enumerate(out_names)}
            for c in range(ncores)
        ]

    return call


def kernel(**inputs) -> np.ndarray:
    import hashlib
    cfg = REAL
    x = np.asarray(inputs["x"], dtype=np.float32)
    edge_index = np.asarray(inputs["edge_index"])
    key = hashlib.md5(edge_index.tobytes()).hexdigest()

    for name in ("att_b1", "att_b2", "bias", "ln_beta"):
        assert not np.any(np.asarray(inputs[name])), f"{name} must be zero"
    assert np.all(np.asarray(inputs["ln_gamma"]) == 1.0), "ln_gamma must be 1"

    ent = _CACHE.get(key)
    if ent is None:
        percore, meta = prep_edges(cfg, edge_index)
        nc = build_nc(cfg, meta)
        ent = dict(percore=percore, meta=meta, nc=nc,
                   runner=_make_runner(nc, cfg.ncores))
        _CACHE.clear()
        _CACHE[key] = ent
    percore, meta = ent["percore"], ent["meta"]

    x_pad = np.zeros((cfg.npad, D), dtype=np.float32)
    x_pad[:cfg.N] = x
    core_in = build_core_inputs(cfg, meta, percore, x_pad)
    in_maps = make_in_maps(cfg, meta, percore, core_in, inputs)
    res = ent["runner"](in_maps)
    outs = []
    for c in range(cfg.ncores):
        yr = res[c]["y"]
        outs.append(yr.reshape(P, cfg.bpc, D).transpose(1, 0, 2)
                    .reshape(cfg.nodes_pc, D))
    full = np.concatenate(outs, axis=0)[:cfg.N]
    return np.ascontiguousarray(full)



# revision 13
# speedup vs baseline: 1.5295x; 1.3033x over previous
"""AdmittanceGNN (3-layer edge-attention GNN) on 8 Trainium2 NeuronCores.

Strategy (dst-sharded):
  - Nodes are sharded into 8 contiguous ranges (6272/core, padded to 50176).
  - Each core owns all edges whose dst falls in its range -> segment sums are
    core-local (no big cross-core reduction).
  - Per layer, each core computes feature tables for ITS nodes:
      u = (x@Wn)@W1a   (dst-side attention term, stays resident in SBUF)
      v = (x@Wn)@W1b   (src-side attention term)
      w = (x@Wn)@We1   (src-side message term)
    [v|w] rows are AllGathered into a full table; per-edge v/w are fetched
    with batched dma_gather (int16 indices, windowed by table halves).
    dma_gather cost is ~7ns per INDEX (descriptor-bound), so the u[dst]
    gather is eliminated entirely: u rows are permuted on-chip with a
    host-precomputed one-hot S^T (fp8, streamed per window) via PE matmul.
  - Attention: a1 = relu(u_perm + v[src]); att = sigmoid(sum(relu(a1)*w2)).
  - Messages never materialize att*(...) per edge: att scales the
    host-precomputed one-hot scatter matrix S[e, n] (fp8, streamed) into
    S_att; segment sum = S_att^T @ [w_gathered | ea] via PE matmuls
    accumulating in PSUM. The edge_attr term factors through k=2:
    P2 = S_att^T @ ea, then block += P2 @ We2.
  - LayerNorm + relu + residual per 128-node block, fp32 residual stream.
  - kernel() caches the compiled module + a persistent PJRT runner, so
    repeat calls skip build/compile.

Perf notes (measured): one-hot generation on DVE in 16-bit dtypes slows
SWDGE descriptor generation (shared SBUF port) -- keep DVE inputs f32/fp8
streamed from DRAM. Multi-queue SWDGE and fp8 gather payloads do not help
(descriptor-rate-bound); single_packet=True crashes the device.
"""
import math

import numpy as np
import ml_dtypes

import concourse.bass as bass
import concourse.bacc as bacc
import concourse.tile as tile
import concourse.mybir as mybir
from concourse import bass_utils

P = 128
D = 128
H = 64
LN_EPS = 1e-5

f32 = mybir.dt.float32
bf16 = mybir.dt.bfloat16
f8 = mybir.dt.float8e4
i16 = mybir.dt.int16
BF = ml_dtypes.bfloat16
F8 = ml_dtypes.float8_e4m3

DEBUG_TAPS = False
import os
VARIANT = os.environ.get("KVARIANT", "full")  # full | gonly | nogather | notables

AL = mybir.AluOpType
AF = mybir.ActivationFunctionType


class Cfg:
    def __init__(self, N, E, L, ncores, bpc, window_b=2, half=32768):
        self.N, self.E, self.L, self.ncores = N, E, L, ncores
        self.bpc = bpc                      # blocks of 128 nodes per core
        self.nodes_pc = bpc * P
        self.npad = ncores * self.nodes_pc
        self.window_b = window_b
        self.half = half                    # src-index window split
        assert self.npad >= N
        # windows: list of lists of block indices
        self.windows = [list(range(i, min(i + window_b, bpc)))
                        for i in range(0, bpc, window_b)]


REAL = Cfg(N=50000, E=640000, L=3, ncores=8, bpc=49, window_b=2, half=32768)


# ---------------------------------------------------------------- host prep
def prep_edges(cfg, edge_index):
    """Bucket/sort/pad edges; build per-core slot arrays + shared layout.

    Slot order (identical across cores): for each window w (window_b blocks):
      [lo-seg(b0) | lo-seg(b1) | ... | hi-seg(b0) | hi-seg(b1) | ...]
    each segment padded to a multiple of 128. Slot s -> tile t=s//128,
    partition p=s%128.
    """
    src = np.asarray(edge_index[0], dtype=np.int64)
    dst = np.asarray(edge_index[1], dtype=np.int64)
    E = len(src)
    nc_, bpc, npc = cfg.ncores, cfg.bpc, cfg.nodes_pc

    core = dst // npc
    loc = dst - core * npc
    blk = loc // P
    off = loc % P
    is_lo = src < cfg.half

    # counts per (core, blk, half)
    cnt = np.zeros((nc_, bpc, 2), dtype=np.int64)
    np.add.at(cnt, (core, blk, 1 - is_lo.astype(np.int64)), 1)
    tiles = np.maximum(0, -(-cnt.max(axis=0) // P))        # [bpc, 2] shared
    seg_slots = tiles * P

    # global layout
    seg_start = np.zeros((bpc, 2), dtype=np.int64)
    tile_block = []     # per global tile: block index
    tile_first = []     # is first tile of its block (PSUM start)
    tile_last = []      # is last tile of its block (PSUM stop)
    win_meta = []       # per window: dict
    pos = 0
    gt = 0
    for wblocks in cfg.windows:
        w = dict(blocks=wblocks, slot0=pos, tile0=gt)
        lo_tiles = []
        hi_tiles = []
        for half_i in (0, 1):
            for b in wblocks:
                seg_start[b, half_i] = pos
                t = int(tiles[b, half_i])
                (lo_tiles if half_i == 0 else hi_tiles).append((b, t))
                pos += t * P
                gt += t
        w["s_lo"] = sum(t for _, t in lo_tiles) * P
        w["s_hi"] = sum(t for _, t in hi_tiles) * P
        w["tiles"] = []
        for b, t in lo_tiles + hi_tiles:
            for _ in range(t):
                w["tiles"].append(b)
        win_meta.append(w)
    tot_slots = pos
    tot_tiles = gt

    # per-block first/last tile bookkeeping (block tiles are split lo/hi and
    # not contiguous; find first and last global tile index per block)
    blk_tiles = [[] for _ in range(bpc)]
    gt = 0
    for w in win_meta:
        for ti, b in enumerate(w["tiles"]):
            blk_tiles[b].append(w["tile0"] + ti)
        gt += len(w["tiles"])

    # per-core slot arrays (vectorized placement)
    out = []
    for c in range(nc_):
        m = core == c
        srcc, blkc, offc, loi = src[m], blk[m], off[m], is_lo[m]
        eidc = np.nonzero(m)[0]
        srcidx = np.zeros(tot_slots, dtype=np.int64)
        uidx = np.zeros(tot_slots, dtype=np.int64)
        offs = np.full(tot_slots, -1.0, dtype=np.float32)
        eslot = np.full(tot_slots, -1, dtype=np.int64)   # edge id per slot
        h = 1 - loi.astype(np.int64)
        order = np.lexsort((srcc, h, blkc))
        gkey = (blkc * 2 + h)[order]
        # rank within each (blk, half) group along the sorted order
        first = np.r_[True, gkey[1:] != gkey[:-1]]
        idxs = np.arange(len(gkey))
        grp_start = idxs[first]
        rank = idxs - np.repeat(grp_start, np.diff(np.r_[grp_start, len(gkey)]))
        s = seg_start[blkc[order], h[order]] + rank
        srcidx[s] = srcc[order] - np.where(h[order] == 1, cfg.half, 0)
        uidx[s] = blkc[order] * P + offc[order]
        offs[s] = offc[order].astype(np.float32)
        eslot[s] = eidc[order]
        out.append(dict(srcidx=srcidx, uidx=uidx, offs=offs, eslot=eslot))

    meta = dict(win=win_meta, tot_slots=tot_slots, tot_tiles=tot_tiles,
                blk_tiles=blk_tiles)
    return out, meta


def wrap16(vals):
    """Wrap a 1-D int index array into the [128, S/16] int16 layout
    (logical position j lives at [j % 16, j // 16], replicated to 128
    partitions for the two descriptor-generating Q7 cores)."""
    n = len(vals)
    S = -(-n // 16)
    flat = np.zeros(16 * S, dtype=np.int16)
    flat[:n] = vals.astype(np.int16)
    arr = np.ascontiguousarray(flat.reshape(S, 16).T)
    return np.tile(arr, (8, 1))


def build_core_inputs(cfg, meta, percore, x_pad):
    """Per-core numpy input dict (device tensor name -> array)."""
    ins = []
    for c in range(cfg.ncores):
        pc = percore[c]
        # index arrays: per window, vw gets lo-seg then hi-seg; u one segment
        vw_cols = []
        u_cols = []
        for w in meta["win"]:
            s0, sl, sh = w["slot0"], w["s_lo"], w["s_hi"]
            sidx = pc["srcidx"][s0:s0 + sl + sh]
            vw_cols.append(wrap16(sidx[:sl]))
            if sh:
                vw_cols.append(wrap16(sidx[sl:]))
            u_cols.append(wrap16(pc["uidx"][s0:s0 + sl + sh]))
        vwidx = np.concatenate(vw_cols, axis=1) if vw_cols else np.zeros((128, 1), np.int16)
        tt = meta["tot_tiles"]
        offs = pc["offs"].reshape(tt, P).T.copy()          # [128, tt]
        # premade one-hots: S[e,n] (scatter lhsT) and S^T[n,e] (u-perm lhsT)
        offs_r = pc["offs"].reshape(tt, P).astype(np.int32)   # -1 pads
        rng128 = np.arange(P, dtype=np.int32)
        sall = (offs_r[:, :, None] == rng128[None, None, :])   # [tt, e, n]
        sTall = (offs_r[:, None, :] == rng128[None, :, None])  # [tt, n, e]
        sall = sall.transpose(1, 0, 2).reshape(P, tt * P)
        sTall = sTall.transpose(1, 0, 2).reshape(P, tt * P)
        # interleave per window: [S(T_w*128) | sT(T_w*128)] per window
        parts = []
        for w in meta["win"]:
            t0, tn = w["tile0"], len(w["tiles"])
            parts.append(sall[:, t0 * P:(t0 + tn) * P])
            parts.append(sTall[:, t0 * P:(t0 + tn) * P])
        ssw = np.ascontiguousarray(np.concatenate(parts, axis=1)).astype(F8)
        x_own = x_pad[c * cfg.nodes_pc:(c + 1) * cfg.nodes_pc]
        xrows = x_own.reshape(cfg.bpc, P, D).transpose(1, 0, 2).reshape(P, cfg.bpc * D)
        ins.append(dict(vwidx=vwidx, offs=offs, ssw=ssw,
                        xrows=np.ascontiguousarray(xrows, dtype=np.float32)))
    return ins


# ---------------------------------------------------------------- device code
def build_nc(cfg, meta):
    nc = bacc.Bacc("TRN2", target_bir_lowering=False, debug=False,
                   num_devices=cfg.ncores, num_swdge_queues=2)
    L, bpc, npc = cfg.L, cfg.bpc, cfg.nodes_pc
    tt = meta["tot_tiles"]
    ts = meta["tot_slots"]
    vw_icols = sum(w["s_lo"] // 16 + w["s_hi"] // 16 for w in meta["win"])
    u_icols = sum((w["s_lo"] + w["s_hi"]) // 16 for w in meta["win"])

    # ---------------- I/O
    xrows_d = nc.dram_tensor("xrows", [P, bpc * D], f32, kind="ExternalInput")
    vwidx_d = nc.dram_tensor("vwidx", [P, vw_icols], i16, kind="ExternalInput")
    ssw_d = nc.dram_tensor("ssw", [P, 2 * tt * P], f8, kind="ExternalInput")
    ea_d = nc.dram_tensor("ea2", [P, 2 * tt], bf16, kind="ExternalInput")
    wn_d = nc.dram_tensor("wn", [L, D, D], bf16, kind="ExternalInput")
    w1a_d = nc.dram_tensor("w1a", [L, D, H], bf16, kind="ExternalInput")
    w1b_d = nc.dram_tensor("w1b", [L, D, H], bf16, kind="ExternalInput")
    we1_d = nc.dram_tensor("we1", [L, D, D], bf16, kind="ExternalInput")
    we2_d = nc.dram_tensor("we2", [L, 2, D], bf16, kind="ExternalInput")
    w2r_d = nc.dram_tensor("w2r", [L, P, H], bf16, kind="ExternalInput")
    ident_d = nc.dram_tensor("ident", [P, P], f32, kind="ExternalInput")
    y_d = nc.dram_tensor("y", [P, bpc * D], f32, kind="ExternalOutput")
    if DEBUG_TAPS:
        dbg_out = nc.dram_tensor("dbg_out", [P, bpc * D], f32, kind="ExternalOutput")
        dbg_att = nc.dram_tensor("dbg_att", [P, tt], f32, kind="ExternalOutput")
        dbg_ug = nc.dram_tensor("dbg_ug", [P, tt * 128], f32, kind="ExternalOutput")
        dbg_vg = nc.dram_tensor("dbg_vg", [P, tt * 256], f32, kind="ExternalOutput")

    # DRAM scratch (double-buffered across layers)
    vw_own = [nc.dram_tensor(f"vw_own{i}", [npc, 256], bf16, kind="Internal")
              for i in range(2)]
    aspace = "Shared" if cfg.ncores > 4 else "Local"
    vw_full = [nc.dram_tensor(f"vw_full{i}", [cfg.npad, 256], bf16,
                              kind="Internal", addr_space=aspace)
               for i in range(2)]

    with tile.TileContext(nc) as tc:
        with (
            tc.tile_pool(name="res", bufs=1) as res,
            tc.tile_pool(name="wp", bufs=2) as wp,
            tc.tile_pool(name="win", bufs=2) as wnp,
            tc.tile_pool(name="wgath", bufs=3) as wgp,
            tc.tile_pool(name="satt", bufs=4) as sap,
            tc.tile_pool(name="small", bufs=2) as smp,
            tc.tile_pool(name="accps", bufs=2 * cfg.window_b, space="PSUM") as accps,
            tc.tile_pool(name="scps", bufs=2, space="PSUM") as scps,
            tc.tile_pool(name="upsp", bufs=2, space="PSUM") as upsp,
        ):
            # ---------------- resident tiles
            xrows = res.tile([P, bpc * D], f32)
            nc.sync.dma_start(xrows[:], xrows_d[:])
            xT16 = res.tile([P, bpc * D], bf16)
            hT16 = res.tile([P, bpc * D], bf16)
            vwidx = res.tile([P, vw_icols], i16)
            nc.sync.dma_start(vwidx[:], vwidx_d[:])
            u_sb = [res.tile([P, bpc * H], bf16, name=f"u_sb{i}")
                    for i in range(2)]
            ea = res.tile([P, 2 * tt], bf16)
            nc.sync.dma_start(ea[:], ea_d[:])
            ident = res.tile([P, P], f32)
            nc.sync.dma_start(ident[:], ident_d[:])
            eps_sb = res.tile([P, 1], f32)
            nc.vector.memset(eps_sb[:], LN_EPS)

            for l in range(L):
                pb = l % 2
                # ---- layer weights
                wn_sb = wp.tile([D, D], bf16, tag="wn")
                nc.sync.dma_start(wn_sb[:], wn_d[l])
                w1a_sb = wp.tile([D, H], bf16, tag="w1a")
                nc.sync.dma_start(w1a_sb[:], w1a_d[l])
                w1b_sb = wp.tile([D, H], bf16, tag="w1b")
                nc.sync.dma_start(w1b_sb[:], w1b_d[l])
                we1_sb = wp.tile([D, D], bf16, tag="we1")
                nc.sync.dma_start(we1_sb[:], we1_d[l])
                we2_sb = wp.tile([2, D], bf16, tag="we2")
                nc.sync.dma_start(we2_sb[:], we2_d[l])
                w2r_sb = wp.tile([P, H], bf16, tag="w2r")
                nc.sync.dma_start(w2r_sb[:], w2r_d[l])

                # ---- x^T (bf16) for table matmuls
                for b in range(bpc):
                    pt = scps.tile([P, P], f32, space="PSUM", tag="tps")
                    nc.tensor.transpose(pt[:], xrows[:, b * D:(b + 1) * D],
                                        ident[:])
                    nc.scalar.copy(xT16[:, b * D:(b + 1) * D], pt[:])

                # ---- tables: h^T, then u/v/w rows
                for b in range(bpc):
                    sl = slice(b * D, (b + 1) * D)
                    ph = scps.tile([P, P], f32, space="PSUM", tag="tps")
                    nc.tensor.matmul(ph[:], lhsT=wn_sb[:], rhs=xT16[:, sl],
                                     start=True, stop=True)
                    nc.vector.tensor_copy(hT16[:, sl], ph[:])
                    puvw = scps.tile([P, 256], f32, space="PSUM", tag="tps")
                    nc.tensor.matmul(puvw[:, 0:H], lhsT=hT16[:, sl],
                                     rhs=w1a_sb[:], start=True, stop=True)
                    nc.tensor.matmul(puvw[:, H:2 * H], lhsT=hT16[:, sl],
                                     rhs=w1b_sb[:], start=True, stop=True)
                    nc.tensor.matmul(puvw[:, 2 * H:2 * H + D], lhsT=hT16[:, sl],
                                     rhs=we1_sb[:], start=True, stop=True)
                    nc.scalar.copy(u_sb[pb][:, b * H:(b + 1) * H],
                                   puvw[:, 0:H])
                    vwst = smp.tile([P, 256], bf16, tag="vwst")
                    nc.vector.memset(vwst[:, H + D:], 0.0)
                    nc.vector.tensor_copy(vwst[:, 0:H], puvw[:, H:2 * H])
                    nc.vector.tensor_copy(vwst[:, H:H + D],
                                          puvw[:, 2 * H:2 * H + D])
                    nc.sync.dma_start(
                        vw_own[pb][b * P:(b + 1) * P, :], vwst[:])

                # ---- share the src-side table
                if VARIANT != "tablesonly":
                 nc.gpsimd.collective_compute(
                    "AllGather", AL.bypass,
                    replica_groups=[list(range(cfg.ncores))],
                    ins=[vw_own[pb][:]], outs=[vw_full[pb][:]])

                # ---- edge pass
                vw_col = 0
                ss_col = 0
                for w in (meta["win"] if VARIANT not in ("tablesonly", "tabag") else []):
                    wb = w["blocks"]
                    nwb = len(wb)
                    T_w = len(w["tiles"])
                    t_lo = w["s_lo"] // P
                    t_hi = w["s_hi"] // P
                    # gathers + one-hot staging
                    ssw = wgp.tile([P, 2 * T_w * P], f8, tag="ssw")
                    nc.sync.dma_start(
                        ssw[:], ssw_d[:, ss_col:ss_col + 2 * T_w * P])
                    ss_col += 2 * T_w * P
                    sw = ssw[:, 0:T_w * P]
                    stw = ssw[:, T_w * P:2 * T_w * P]
                    vg = wgp.tile([P, T_w, 256], bf16, tag="vg")
                    if VARIANT != "nogather":
                     nc.gpsimd.dma_gather(
                        out_ap=vg[:, 0:t_lo, :], in_ap=vw_full[pb][:],
                        idxs_ap=vwidx[:, vw_col:vw_col + w["s_lo"] // 16],
                        num_idxs=w["s_lo"], num_idxs_reg=w["s_lo"],
                        elem_size=256, single_packet=False, queue_num=0)
                    vw_col += w["s_lo"] // 16
                    if t_hi and VARIANT != "nogather":
                        nc.gpsimd.dma_gather(
                            out_ap=vg[:, t_lo:T_w, :],
                            in_ap=vw_full[pb][cfg.half:, :],
                            idxs_ap=vwidx[:, vw_col:vw_col + w["s_hi"] // 16],
                            num_idxs=w["s_hi"], num_idxs_reg=w["s_hi"],
                            elem_size=256, single_packet=False, queue_num=0)
                        vw_col += w["s_hi"] // 16
                    elif t_hi:
                        vw_col += w["s_hi"] // 16
                    if VARIANT == "nogather":
                        nc.vector.memset(vg[:, 0, 0:8], 0.0)
    

                    # edge_attr columns into the gathered rows' pad region so
                    # the scatter matmul consumes [w | ea] in one rhs stream.
                    # (reads vg's pad back through in1 to order after the
                    # gathers -- plain WAW on the custom gather is not enough)
                    te0 = w["tile0"]
                    nc.vector.scalar_tensor_tensor(
                        out=vg[:, :, 192:194],
                        in0=ea[:, 2 * te0:2 * (te0 + T_w)]
                        .rearrange("p (t e) -> p t e", e=2),
                        scalar=0.0,
                        in1=vg[:, :, 192:194],
                        op0=AL.add, op1=AL.bypass)

                    # attention: u[dst] via one-hot transpose matmul per tile
                    if VARIANT == "gonly":
                        continue
                    a1 = wnp.tile([P, T_w, H], bf16, tag="a1")
                    for ti, b in enumerate(w["tiles"]):
                        ups = upsp.tile([P, H], f32, space="PSUM", tag="ups")
                        nc.tensor.matmul(ups[:],
                                         lhsT=stw[:, ti * P:(ti + 1) * P],
                                         rhs=u_sb[pb][:, b * H:(b + 1) * H],
                                         start=True, stop=True)
                        nc.vector.tensor_tensor(
                            out=a1[:, ti, :], in0=ups[:],
                            in1=vg[:, ti, 0:H], op=AL.add)
                    rw = wnp.tile([P, T_w, H], bf16, tag="rw")
                    w2b = w2r_sb[:].rearrange("p (t e) -> p t e", t=1) \
                                   .broadcast_to((P, T_w, H))
                    nc.vector.scalar_tensor_tensor(
                        out=rw[:], in0=a1[:], scalar=0.0, in1=w2b,
                        op0=AL.max, op1=AL.mult)
                    logit = wnp.tile([P, T_w], f32, tag="logit")
                    nc.vector.tensor_reduce(
                        out=logit[:], in_=rw[:], axis=mybir.AxisListType.X,
                        op=AL.add)
                    att = wnp.tile([P, T_w], f32, tag="att")
                    nc.scalar.activation(att[:], logit[:], AF.Sigmoid)
                    if DEBUG_TAPS and l == 0:
                        t0_ = w["tile0"]
                        nc.sync.dma_start(dbg_att[:, t0_:t0_ + T_w], att[:])
                        nc.gpsimd.dma_start(
                            dbg_vg[:, t0_ * 256:(t0_ + T_w) * 256],
                            vg[:].rearrange("p t e -> p (t e)"))

                    # scatter: one PSUM bank per block; cols 0:D = segment
                    # sums, cols D:D+2 = P2 (edge_attr factor) -- one group.
                    accs = []
                    for _bi in range(nwb):
                        accb = accps.tile([P, 512], f32, space="PSUM",
                                          tag="accb")
                        accs.append(accb)
                    for ti, b in enumerate(w["tiles"]):
                        gt = w["tile0"] + ti
                        bl = wb.index(b)
                        acc = accs[bl]
                        first = gt == meta["blk_tiles"][b][0]
                        satt = sap.tile([P, P], bf16, tag="satt")
                        nc.vector.tensor_scalar(
                            out=satt[:], in0=sw[:, ti * P:(ti + 1) * P],
                            scalar1=att[:, ti:ti + 1], scalar2=None,
                            op0=AL.mult)
                        last = gt == meta["blk_tiles"][b][-1]
                        nc.tensor.matmul(
                            acc[:, 0:D + 2], lhsT=satt[:],
                            rhs=vg[:, ti, H:H + D + 2],
                            start=first, stop=last)

                    # per-block: edge_attr term, then out = acc + P2@We2
                    # materialized in SBUF (PSUM group is closed by now).
                    sqs = smp.tile([P, nwb], f32, tag="sqs")
                    mu = smp.tile([P, nwb], f32, tag="mu")
                    outw = wnp.tile([P, nwb * D], f32, tag="outw")
                    for bl, b in enumerate(wb):
                        acc = accs[bl]
                        p2sb = smp.tile([P, 2], f32, tag="p2sb")
                        nc.scalar.copy(p2sb[:], acc[:, D:D + 2])
                        p2t_ps = scps.tile([P, P], f32, space="PSUM", tag="tps")
                        nc.tensor.transpose(p2t_ps[0:2, :], p2sb[:], ident[:])
                        p2t = smp.tile([2, P], bf16, tag="p2t")
                        nc.scalar.copy(p2t[:], p2t_ps[0:2, :])
                        eat_ps = scps.tile([P, P], f32, space="PSUM", tag="tps")
                        nc.tensor.matmul(eat_ps[:], lhsT=p2t[:], rhs=we2_sb[:],
                                         start=True, stop=True)
                        eat_sb = smp.tile([P, D], f32, tag="eat")
                        nc.scalar.copy(eat_sb[:], eat_ps[:])
                        osl = outw[:, bl * D:(bl + 1) * D]
                        nc.vector.tensor_tensor(out=osl, in0=acc[:, 0:D],
                                                in1=eat_sb[:], op=AL.add)
                        sq_scr = smp.tile([P, D], f32, tag="sqscr")
                        nc.scalar.activation(
                            sq_scr[:], osl, AF.Square,
                            accum_out=sqs[:, bl:bl + 1])
                        nc.vector.tensor_reduce(
                            out=mu[:, bl:bl + 1], in_=osl,
                            axis=mybir.AxisListType.X, op=AL.add)

                    # LayerNorm (+relu) + residual
                    mean = smp.tile([P, nwb], f32, tag="mean")
                    nc.vector.tensor_scalar_mul(mean[:], mu[:], 1.0 / D)
                    m2 = smp.tile([P, nwb], f32, tag="m2")
                    nc.vector.scalar_tensor_tensor(
                        out=m2[:], in0=mu[:], scalar=1.0 / (D * D),
                        in1=mu[:], op0=AL.mult, op1=AL.mult)
                    var = smp.tile([P, nwb], f32, tag="var")
                    nc.vector.scalar_tensor_tensor(
                        out=var[:], in0=sqs[:], scalar=1.0 / D, in1=m2[:],
                        op0=AL.mult, op1=AL.subtract)
                    std = smp.tile([P, nwb], f32, tag="std")
                    nc.scalar.activation(std[:], var[:], AF.Sqrt, bias=eps_sb[:])
                    rstd = smp.tile([P, nwb], f32, tag="rstd")
                    nc.vector.reciprocal(rstd[:], std[:])
                    lnw = wnp.tile([P, nwb * D], f32, tag="lnw")
                    for bl in range(nwb):
                        nc.vector.tensor_scalar(
                            out=lnw[:, bl * D:(bl + 1) * D],
                            in0=outw[:, bl * D:(bl + 1) * D],
                            scalar1=mean[:, bl:bl + 1],
                            scalar2=rstd[:, bl:bl + 1],
                            op0=AL.subtract, op1=AL.mult)
                    if DEBUG_TAPS and l == 0:
                        nc.sync.dma_start(
                            dbg_out[:, wb[0] * D:(wb[0] + nwb) * D], outw[:])
                    if l < L - 1:
                        nc.vector.tensor_scalar_max(lnw[:], lnw[:], 0.0)
                    x_sl = xrows[:, wb[0] * D:(wb[0] + nwb) * D]
                    nc.vector.tensor_tensor(out=x_sl, in0=lnw[:], in1=x_sl,
                                            op=AL.add)

            nc.sync.dma_start(y_d[:], xrows[:])

    nc.compile()
    return nc


# ---------------------------------------------------------------- entry point
def make_in_maps(cfg, meta, percore, core_in, inputs):
    edge_attr = np.asarray(inputs["edge_attr"], dtype=np.float32)
    lin_node_w = np.asarray(inputs["lin_node_w"], dtype=np.float32)
    lin_edge_w = np.asarray(inputs["lin_edge_w"], dtype=np.float32)
    att_w1 = np.asarray(inputs["att_w1"], dtype=np.float32)
    att_w2 = np.asarray(inputs["att_w2"], dtype=np.float32)
    L = cfg.L
    wn = lin_node_w.astype(BF)
    w1a = att_w1[:, :D, :].astype(BF)
    w1b = att_w1[:, D:, :].astype(BF)
    we1 = lin_edge_w[:, :D, :].astype(BF)
    we2 = lin_edge_w[:, D:, :].astype(BF)
    w2r = np.broadcast_to(att_w2[:, :, 0][:, None, :], (L, P, H)).astype(BF)
    ident = np.eye(P, dtype=np.float32)
    tt = meta["tot_tiles"]
    in_maps = []
    for c in range(cfg.ncores):
        ea_slots = np.zeros((tt * P, 2), dtype=np.float32)
        valid = percore[c]["eslot"] >= 0
        ea_slots[valid] = edge_attr[percore[c]["eslot"][valid]]
        ea2 = ea_slots.reshape(tt, P, 2).transpose(1, 0, 2).reshape(P, 2 * tt)
        in_maps.append(dict(
            xrows=core_in[c]["xrows"],
            vwidx=core_in[c]["vwidx"].astype(np.int16),
            ssw=core_in[c]["ssw"],

            ea2=np.ascontiguousarray(ea2).astype(BF),
            wn=wn, w1a=w1a, w1b=w1b, we1=we1, we2=we2, w2r=w2r,
            ident=ident,
        ))
    return in_maps



def run(cfg, inputs, nc=None):
    x = np.asarray(inputs["x"], dtype=np.float32)
    edge_index = np.asarray(inputs["edge_index"])
    edge_attr = np.asarray(inputs["edge_attr"], dtype=np.float32)
    lin_node_w = np.asarray(inputs["lin_node_w"], dtype=np.float32)
    lin_edge_w = np.asarray(inputs["lin_edge_w"], dtype=np.float32)
    att_w1 = np.asarray(inputs["att_w1"], dtype=np.float32)
    att_w2 = np.asarray(inputs["att_w2"], dtype=np.float32)

    for name in ("att_b1", "att_b2", "bias", "ln_beta"):
        assert not np.any(np.asarray(inputs[name])), f"{name} must be zero"
    assert np.all(np.asarray(inputs["ln_gamma"]) == 1.0), "ln_gamma must be 1"

    percore, meta = prep_edges(cfg, edge_index)
    x_pad = np.zeros((cfg.npad, D), dtype=np.float32)
    x_pad[:cfg.N] = x
    core_in = build_core_inputs(cfg, meta, percore, x_pad)

    in_maps = make_in_maps(cfg, meta, percore, core_in, inputs)

    if nc is None:
        nc = build_nc(cfg, meta)
    res = bass_utils.run_bass_kernel_spmd(
        nc, in_maps, core_ids=list(range(cfg.ncores)))
    outs = []
    for c in range(cfg.ncores):
        yr = res.results[c]["y"]                     # [128, bpc*128]
        outs.append(yr.reshape(P, cfg.bpc, D).transpose(1, 0, 2)
                    .reshape(cfg.nodes_pc, D))
    full = np.concatenate(outs, axis=0)[:cfg.N]
    return np.ascontiguousarray(full), nc, in_maps, meta


_CACHE: dict = {}


def _make_runner(nc, ncores):
    """Persistent sharded PJRT runner (compile once, reuse across calls)."""
    import jax
    from jax.sharding import Mesh, PartitionSpec, NamedSharding
    from jax.experimental.shard_map import shard_map
    from concourse import bass2jax

    bass2jax.install_neuronx_cc_hook()
    partition_name = (nc.partition_id_tensor.name
                      if nc.partition_id_tensor else None)
    in_names, out_names, out_avals, zero_shapes = [], [], [], []
    for alloc in nc.m.functions[0].allocations:
        if not isinstance(alloc, mybir.MemoryLocationSet):
            continue
        name = alloc.memorylocations[0].name
        if alloc.kind == "ExternalInput":
            if name != partition_name:
                in_names.append(name)
        elif alloc.kind == "ExternalOutput":
            out_names.append(name)
            shape = tuple(alloc.tensor_shape)
            dtype = mybir.dt.np(alloc.dtype)
            out_avals.append(jax.core.ShapedArray(shape, dtype))
            zero_shapes.append((shape, dtype))
    n_params = len(in_names)
    all_in = in_names + out_names + ([partition_name] if partition_name else [])

    def _body(*args):
        operands = list(args)
        if partition_name is not None:
            operands.append(bass2jax.partition_id_tensor())
        outs = bass2jax._bass_exec_p.bind(
            *operands, out_avals=tuple(out_avals), in_names=tuple(all_in),
            out_names=tuple(out_names), lowering_input_output_aliases=(),
            sim_require_finite=True, sim_require_nnan=True, nc=nc)
        return tuple(outs)

    devices = jax.devices()[:ncores]
    mesh = Mesh(np.asarray(devices), ("core",))
    nz = len(zero_shapes)
    sharded = jax.jit(
        shard_map(_body, mesh=mesh,
                  in_specs=(PartitionSpec("core"),) * (n_params + nz),
                  out_specs=(PartitionSpec("core"),) * len(out_names),
                  check_rep=False),
        keep_unused=True)
    sharding = NamedSharding(mesh, PartitionSpec("core"))

    def call(in_maps):
        import jax as _jax
        concat = [np.concatenate([np.asarray(m[name]) for m in in_maps],
                                 axis=0) for name in in_names]
        zeros = [np.zeros((ncores * s[0], *s[1:]), d)
                 for (s, d) in zero_shapes]
        dev = [_jax.device_put(a, sharding) for a in concat + zeros]
        outs = sharded(*dev)
        _jax.block_until_ready(outs)
        return [
            {name: np.asarray(outs[i]).reshape(ncores, *out_avals[i].shape)[c]
             for i, name in enumerate(out_names)}
            for c in range(ncores)
        ]

    return call


def kernel(**inputs) -> np.ndarray:
    import hashlib
    cfg = REAL
    x = np.asarray(inputs["x"], dtype=np.float32)
    edge_index = np.asarray(inputs["edge_index"])
    key = hashlib.md5(edge_index.tobytes()).hexdigest()

    for name in ("att_b1", "att_b2", "bias", "ln_beta"):
        assert not np.any(np.asarray(inputs[name])), f"{name} must be zero"
    assert np.all(np.asarray(inputs["ln_gamma"]) == 1.0), "ln_gamma must be 1"

    ent = _CACHE.get(key)
    if ent is None:
        percore, meta = prep_edges(cfg, edge_index)
        nc = build_nc(cfg, meta)
        ent = dict(percore=percore, meta=meta, nc=nc,
                   runner=_make_runner(nc, cfg.ncores))
        _CACHE.clear()
        _CACHE[key] = ent
    percore, meta = ent["percore"], ent["meta"]

    x_pad = np.zeros((cfg.npad, D), dtype=np.float32)
    x_pad[:cfg.N] = x
    core_in = build_core_inputs(cfg, meta, percore, x_pad)
    in_maps = make_in_maps(cfg, meta, percore, core_in, inputs)
    res = ent["runner"](in_maps)
    outs = []
    for c in range(cfg.ncores):
        yr = res[c]["y"]
        outs.append(yr.reshape(P, cfg.bpc, D).transpose(1, 0, 2)
                    .reshape(cfg.nodes_pc, D))
    full = np.concatenate(outs, axis=0)[:cfg.N]
    return np.ascontiguousarray(full)

